# revision 11
# baseline (speedup 1.0000x reference)
"""Trainium2 Bass kernel for nn_DeeperHyperbolicEncoder.

Math (per batch row r; D_in=512, D_h=256, D_out=128):
  v   = x @ W1^T                 layer-1 matmul (+ fused v.b1 column)
  g   = beta*v + gamma*b1        mobius_add(expmap0(v), b1) collapsed to
                                 per-row scalars from s1=|v|^2, dot=v.b1
  u   = tanh(sb*v + sg*b1)       project+logmap0+tanh folded into row scalars
  q   = u @ W2^T                 (+ fused q.b2 column; mobius_matvec(W2, expmap0(u))
                                 == expmap0(u @ W2^T))
  out = pb*q + pg*b2             mobius_add + double-project via analytic norms

Precision: layer-1 matmul runs as a 3-term fp32r split (x_hi@W_hi + x_hi@W_lo
+ x_lo_bf16@W_bf16) which is exact to ~2^-21; fp32r (11-bit mantissa) streams
at 1 cyc/row vs 4 for fp32. Layer-2 matmul is plain fp32 (its operand u is
produced on device and cannot be cheaply hi/lo split).

Per-row scalar chains are batched across T row-tiles as [128, T] wides.
Data-parallel across 8 NeuronCores (batch split), weights replicated.
"""

import numpy as np
import ml_dtypes

import concourse.bass as bass
import concourse.tile as tile
from concourse import bacc, mybir
from concourse.bass_utils import run_bass_kernel_spmd

F32 = mybir.dt.float32
F32R = mybir.dt.float32r
BF16 = mybir.dt.bfloat16
AF = mybir.ActivationFunctionType
OP = mybir.AluOpType

EPS = 1e-15
MAXN = 1.0 - 4e-3

P = 128
D_IN = 512
D_H = 256
D_OUT = 128
N_CORES = 8


def build_program(nt: int, T: int) -> bass.Bass:
    assert nt % T == 0
    n_sb = nt // T

    nc = bacc.Bacc("TRN2", target_bir_lowering=False, debug=False)

    NW = 260   # layer-1 moving width: 256 outputs + dot col + 3 pad (fp32r needs N%4==0)
    NB = 5168  # packed byte-constants per partition

    xt = nc.dram_tensor("xt", [nt, P, 4, P], F32R, kind="ExternalInput").ap()
    xlo = nc.dram_tensor("xlo", [nt, P, 4, P], BF16, kind="ExternalInput").ap()
    w1r = nc.dram_tensor("w1r", [2, 4, P, NW], F32R, kind="ExternalInput").ap()
    cpk = nc.dram_tensor("cpk", [P, NB], mybir.dt.uint8, kind="ExternalInput").ap()
    out = nc.dram_tensor("out", [nt * P, D_OUT], F32, kind="ExternalOutput").ap()

    with tile.TileContext(nc) as tc:
        from contextlib import ExitStack

        with ExitStack() as ctx:
            _body(ctx, tc, nt, T, n_sb, xt, xlo, w1r, cpk, NW, NB, out)
    nc.compile()
    return nc


def _body(ctx, tc, nt, T, n_sb, xt, xlo, w1r, cpk, NW, NB, out):
    nc = tc.nc

    cpool = ctx.enter_context(tc.tile_pool(name="cpool", bufs=1))
    w1r_sb = cpool.tile([P, 2, 4, NW], F32R, name="w1r_sb")
    nc.sync.dma_start(w1r_sb[:], w1r.rearrange("h k p n -> p h k n"))
    w1hi_sb = w1r_sb[:, 0]
    w1lo_sb = w1r_sb[:, 1]
    cpk_sb = cpool.tile([P, NB], mybir.dt.uint8, name="cpk_sb")
    nc.sync.dma_start(cpk_sb[:], cpk[:])
    w1b_sb = cpk_sb[:, 0:2080].bitcast(BF16).rearrange("p (k n) -> p k n", k=4)
    w2_sb = cpk_sb[:, 2080:3112].bitcast(F32).rearrange("p (k n) -> p k n", k=2)
    b1_sb = cpk_sb[:, 3112:4136].bitcast(F32)
    b2_sb = cpk_sb[:, 4136:4648].bitcast(F32)
    id_sb = cpk_sb[:, 4648:5160].bitcast(F32)
    cst = cpk_sb[:, 5160:5168].bitcast(F32)
    y1 = cst[:, 0:1]
    y2 = cst[:, 1:2]

    xpool = ctx.enter_context(tc.tile_pool(name="xpool", bufs=3))
    vwpool = ctx.enter_context(tc.tile_pool(name="vwpool", bufs=2))
    qwpool = ctx.enter_context(tc.tile_pool(name="qwpool", bufs=2))
    scpool = ctx.enter_context(tc.tile_pool(name="scpool", bufs=2))
    gpool = ctx.enter_context(tc.tile_pool(name="gpool", bufs=3))
    upool = ctx.enter_context(tc.tile_pool(name="upool", bufs=3))
    utpool = ctx.enter_context(tc.tile_pool(name="utpool", bufs=3))
    opool = ctx.enter_context(tc.tile_pool(name="opool", bufs=4))
    pvpool = ctx.enter_context(tc.tile_pool(name="pvpool", bufs=3, space="PSUM"))
    ptpool = ctx.enter_context(tc.tile_pool(name="ptpool", bufs=2, space="PSUM"))
    pqpool = ctx.enter_context(tc.tile_pool(name="pqpool", bufs=2, space="PSUM"))

    for sb in range(n_sb):
        vw = vwpool.tile([P, T, D_H + 1], F32, name="vw")
        qw = qwpool.tile([P, T, D_OUT + 1], F32, name="qw")
        s1w = scpool.tile([P, T], F32, name="s1w")
        sqw = scpool.tile([P, T], F32, name="sqw")

        # ---------------- phase A: load, mm1 (3-term), evacuate, reduce ----
        for t in range(T):
            ti = sb * T + t
            xsb = xpool.tile([P, 4, P], F32R, name="xsb")
            nc.sync.dma_start(xsb[:], xt[ti])
            xlsb = xpool.tile([P, 4, P], BF16, name="xlsb")
            nc.sync.dma_start(xlsb[:], xlo[ti])
            pv = pvpool.tile([P, NW], F32, name="pv")
            nmm = 0
            for wsb, xop in ((w1hi_sb, xsb), (w1lo_sb, xsb), (w1b_sb, xlsb)):
                for k in range(4):
                    nc.tensor.matmul(
                        pv[:],
                        xop[:, k, :],
                        wsb[:, k, :],
                        start=(nmm == 0),
                        stop=(nmm == 11),
                    )
                    nmm += 1
            nc.scalar.activation(vw[:, t, :], pv[:, : D_H + 1], AF.Copy)
            nc.scalar.activation(
                pv[:, :D_H], pv[:, :D_H], AF.Square, accum_out=s1w[:, t : t + 1]
            )

        # ---------------- chain A: layer-1 per-row scalars -----------------
        dotw = vw[:, :, D_H]

        def st(name):
            return scpool.tile([P, T], F32, name=name)

        n1 = st("n1")
        nc.scalar.activation(n1[:], s1w[:], AF.Sqrt)
        n1c = st("n1c")
        nc.vector.tensor_scalar(n1c[:], n1[:], EPS, None, op0=OP.max)
        rn1 = st("rn1")
        nc.vector.reciprocal(rn1[:], n1c[:])
        th = st("th")
        nc.scalar.activation(th[:], n1c[:], AF.Tanh)
        a1 = st("a1")
        nc.vector.tensor_tensor(a1[:], th[:], rn1[:], op=OP.mult)
        xy = st("xy")
        nc.vector.tensor_tensor(xy[:], a1[:], dotw, op=OP.mult)
        z = st("z")
        nc.vector.tensor_scalar(z[:], xy[:], 2.0, 1.0, op0=OP.mult, op1=OP.add)
        unum = st("unum")
        nc.vector.tensor_scalar(unum[:], z[:], y1, None, op0=OP.add)
        x2 = st("x2")
        nc.vector.tensor_tensor(x2[:], th[:], th[:], op=OP.mult)
        den = st("den")
        nc.vector.scalar_tensor_tensor(den[:], x2[:], y1, z[:], op0=OP.mult, op1=OP.add)
        rden = st("rden")
        nc.vector.reciprocal(rden[:], den[:])
        bta = st("bta")
        nc.vector.tensor_tensor(bta[:], unum[:], rden[:], op=OP.mult)
        beta = st("beta")
        nc.vector.tensor_tensor(beta[:], bta[:], a1[:], op=OP.mult)
        omx2 = st("omx2")
        nc.vector.tensor_scalar(omx2[:], x2[:], -1.0, 1.0, op0=OP.mult, op1=OP.add)
        gam = st("gam")
        nc.vector.tensor_tensor(gam[:], omx2[:], rden[:], op=OP.mult)
        sa = st("sa")
        nc.vector.tensor_tensor(sa[:], beta[:], s1w[:], op=OP.mult)
        sb2 = st("sb2")
        nc.vector.tensor_tensor(sb2[:], gam[:], dotw, op=OP.mult)
        sc_ = st("sc_")
        nc.vector.scalar_tensor_tensor(
            sc_[:], sb2[:], 2.0, sa[:], op0=OP.mult, op1=OP.add
        )
        sd = st("sd")
        nc.vector.tensor_tensor(sd[:], sc_[:], beta[:], op=OP.mult)
        ge = st("ge")
        nc.vector.tensor_tensor(ge[:], gam[:], gam[:], op=OP.mult)
        s2 = st("s2")
        nc.vector.scalar_tensor_tensor(s2[:], ge[:], y1, sd[:], op0=OP.mult, op1=OP.add)
        n2 = st("n2")
        nc.scalar.activation(n2[:], s2[:], AF.Sqrt)
        m_ = st("m_")
        nc.vector.tensor_scalar(m_[:], n2[:], MAXN, None, op0=OP.min)
        rn2 = st("rn2")
        nc.vector.reciprocal(rn2[:], n2[:])
        onep = st("onep")
        nc.vector.tensor_scalar(onep[:], m_[:], 1.0, None, op0=OP.add)
        onem = st("onem")
        nc.vector.tensor_scalar(onem[:], m_[:], -1.0, 1.0, op0=OP.mult, op1=OP.add)
        rom = st("rom")
        nc.vector.reciprocal(rom[:], onem[:])
        rat = st("rat")
        nc.vector.tensor_tensor(rat[:], onep[:], rom[:], op=OP.mult)
        lg = st("lg")
        nc.scalar.activation(lg[:], rat[:], AF.Ln)
        lp = st("lp")
        nc.vector.scalar_tensor_tensor(
            lp[:], lg[:], 0.5, rn2[:], op0=OP.mult, op1=OP.mult
        )
        sbw = st("sbw")
        nc.vector.tensor_tensor(sbw[:], lp[:], beta[:], op=OP.mult)
        sgw = st("sgw")
        nc.vector.tensor_tensor(sgw[:], lp[:], gam[:], op=OP.mult)

        # ---------------- phase B: u = tanh(sb*v + sg*b1); transpose; mm2 --
        for t0 in range(0, T, 2):
            us = []
            for t in (t0, t0 + 1):
                gt = gpool.tile([P, D_H], F32, name="gt")
                nc.vector.tensor_scalar(
                    gt[:], vw[:, t, :D_H], sbw[:, t : t + 1], None, op0=OP.mult
                )
                zt = gpool.tile([P, D_H], F32, name="zt")
                nc.vector.scalar_tensor_tensor(
                    zt[:], b1_sb, sgw[:, t : t + 1], gt[:], op0=OP.mult, op1=OP.add
                )
                ut_ = upool.tile([P, D_H], F32, name="ut_")
                nc.scalar.activation(ut_[:], zt[:], AF.Tanh)
                us.append(ut_)
            ptr = ptpool.tile([P, 4 * P], F32, name="ptr")
            for j, (ui, k) in enumerate([(0, 0), (0, 1), (1, 0), (1, 1)]):
                nc.tensor.transpose(
                    ptr[:, j * P : (j + 1) * P],
                    us[ui][:, k * P : (k + 1) * P],
                    id_sb,
                )
            utt = utpool.tile([P, 4 * P], F32, name="utt")
            nc.vector.tensor_copy(utt[:], ptr[:])
            pq = pqpool.tile([P, 2, D_OUT + 1], F32, name="pq")
            for i in range(2):
                for k in range(2):
                    nc.tensor.matmul(
                        pq[:, i, :],
                        utt[:, (2 * i + k) * P : (2 * i + k + 1) * P],
                        w2_sb[:, k, :],
                        start=(k == 0),
                        stop=(k == 1),
                    )
            nc.vector.tensor_copy(qw[:, t0 : t0 + 2, :], pq[:])
            for i, t in enumerate((t0, t0 + 1)):
                nc.scalar.activation(
                    pq[:, i, :D_OUT],
                    pq[:, i, :D_OUT],
                    AF.Square,
                    accum_out=sqw[:, t : t + 1],
                )

        # ---------------- chain C: layer-2 per-row scalars -----------------
        dot2w = qw[:, :, D_OUT]
        nq = st("nq")
        nc.scalar.activation(nq[:], sqw[:], AF.Sqrt)
        nqc = st("nqc")
        nc.vector.tensor_scalar(nqc[:], nq[:], EPS, None, op0=OP.max)
        rq = st("rq")
        nc.vector.reciprocal(rq[:], nqc[:])
        thq = st("thq")
        nc.scalar.activation(thq[:], nqc[:], AF.Tanh)
        aq = st("aq")
        nc.vector.tensor_tensor(aq[:], thq[:], rq[:], op=OP.mult)
        xy2 = st("xy2")
        nc.vector.tensor_tensor(xy2[:], aq[:], dot2w, op=OP.mult)
        z2 = st("z2")
        nc.vector.tensor_scalar(z2[:], xy2[:], 2.0, 1.0, op0=OP.mult, op1=OP.add)
        unum2 = st("unum2")
        nc.vector.tensor_scalar(unum2[:], z2[:], y2, None, op0=OP.add)
        x22 = st("x22")
        nc.vector.tensor_tensor(x22[:], thq[:], thq[:], op=OP.mult)
        den2 = st("den2")
        nc.vector.scalar_tensor_tensor(
            den2[:], x22[:], y2, z2[:], op0=OP.mult, op1=OP.add
        )
        rden2 = st("rden2")
        nc.vector.reciprocal(rden2[:], den2[:])
        b2a = st("b2a")
        nc.vector.tensor_tensor(b2a[:], unum2[:], rden2[:], op=OP.mult)
        b2c = st("b2c")
        nc.vector.tensor_tensor(b2c[:], b2a[:], aq[:], op=OP.mult)
        omx22 = st("omx22")
        nc.vector.tensor_scalar(omx22[:], x22[:], -1.0, 1.0, op0=OP.mult, op1=OP.add)
        g2c = st("g2c")
        nc.vector.tensor_tensor(g2c[:], omx22[:], rden2[:], op=OP.mult)
        sa2 = st("sa2")
        nc.vector.tensor_tensor(sa2[:], b2c[:], sqw[:], op=OP.mult)
        sb3 = st("sb3")
        nc.vector.tensor_tensor(sb3[:], g2c[:], dot2w, op=OP.mult)
        sc3 = st("sc3")
        nc.vector.scalar_tensor_tensor(
            sc3[:], sb3[:], 2.0, sa2[:], op0=OP.mult, op1=OP.add
        )
        sd2 = st("sd2")
        nc.vector.tensor_tensor(sd2[:], sc3[:], b2c[:], op=OP.mult)
        ge2 = st("ge2")
        nc.vector.tensor_tensor(ge2[:], g2c[:], g2c[:], op=OP.mult)
        np2 = st("np2")
        nc.vector.scalar_tensor_tensor(
            np2[:], ge2[:], y2, sd2[:], op0=OP.mult, op1=OP.add
        )
        npre = st("npre")
        nc.scalar.activation(npre[:], np2[:], AF.Sqrt)
        rnp = st("rnp")
        nc.vector.reciprocal(rnp[:], npre[:])
        pi_ = st("pi_")
        nc.vector.tensor_scalar(pi_[:], rnp[:], MAXN, 1.0, op0=OP.mult, op1=OP.min)
        pb2 = st("pb2")
        nc.vector.tensor_tensor(pb2[:], pi_[:], b2c[:], op=OP.mult)
        pg2 = st("pg2")
        nc.vector.tensor_tensor(pg2[:], pi_[:], g2c[:], op=OP.mult)

        # ---------------- phase D: final combine + store -------------------
        for t in range(T):
            ti = sb * T + t
            o1 = opool.tile([P, D_OUT], F32, name="o1")
            nc.vector.tensor_scalar(
                o1[:], qw[:, t, :D_OUT], pb2[:, t : t + 1], None, op0=OP.mult
            )
            o2 = opool.tile([P, D_OUT], F32, name="o2")
            nc.vector.scalar_tensor_tensor(
                o2[:], b2_sb, pg2[:, t : t + 1], o1[:], op0=OP.mult, op1=OP.add
            )
            nc.sync.dma_start(out[ti * P : (ti + 1) * P, :], o2[:])


def _round_fp32r(a):
    u = np.ascontiguousarray(a, dtype=np.float32).view(np.uint32)
    lsb = (u >> 12) & 1
    rounded = u + 0x7FF + lsb
    return (rounded & 0xFFFFF000).view(np.float32)


def _prep_host(x, W1, b1, W2, b2, n_cores, nt):
    B = x.shape[0]
    assert B == n_cores * nt * P

    W1d = W1.T.astype(np.float64)
    b1d = b1.astype(np.float64)
    W2d = W2.T.astype(np.float64)
    b2d = b2.astype(np.float64)

    NW = 260
    w1ta = np.zeros((D_IN, NW), dtype=np.float32)
    w1ta[:, :D_H] = W1.T.astype(np.float32)
    w1ta[:, D_H] = (W1d @ b1d).astype(np.float32)
    w1hi = _round_fp32r(w1ta)
    w1lo = _round_fp32r(w1ta - w1hi)
    # w1r: [2(hi/lo), 4, P, NW] fp32r
    w1r = np.stack([w1hi.reshape(4, P, NW), w1lo.reshape(4, P, NW)], axis=0)
    w1r = np.ascontiguousarray(w1r)

    # byte-packed constants, laid out per partition: w1b(bf16) | w2tp(f32) |
    # b1f | b2f | ident | [y1, y2]
    w1bf = w1ta.astype(ml_dtypes.bfloat16).reshape(4, P, NW)
    w1bf_p = np.ascontiguousarray(w1bf.transpose(1, 0, 2)).view(np.uint8)
    w1bf_p = w1bf_p.reshape(P, -1)
    w2tp = np.concatenate(
        [W2.T.astype(np.float32), (W2d @ b2d).astype(np.float32)[:, None]], axis=1
    ).reshape(2, P, D_OUT + 1)
    w2tp_p = np.ascontiguousarray(w2tp.transpose(1, 0, 2)).view(np.uint8)
    w2tp_p = w2tp_p.reshape(P, -1)
    b1f = np.ascontiguousarray(np.broadcast_to(b1, (P, D_H)), dtype=np.float32)
    b2f = np.ascontiguousarray(np.broadcast_to(b2, (P, D_OUT)), dtype=np.float32)
    identf = np.eye(P, dtype=np.float32)
    consts = np.zeros((P, 2), dtype=np.float32)
    consts[:, 0] = np.float32(b1d @ b1d)
    consts[:, 1] = np.float32(b2d @ b2d)
    cpk = np.concatenate(
        [
            w1bf_p,
            w2tp_p,
            b1f.view(np.uint8).reshape(P, -1),
            b2f.view(np.uint8).reshape(P, -1),
            identf.view(np.uint8).reshape(P, -1),
            consts.view(np.uint8).reshape(P, -1),
        ],
        axis=1,
    )
    assert cpk.shape == (P, 5168), cpk.shape

    # x -> [core, tile, f(128), k(4), b(128)] transposed blocks; hi in fp32r,
    # residual in bf16
    xr = x.reshape(n_cores, nt, P, 4, P)                   # [c, t, b, k, f]
    xr = np.ascontiguousarray(xr.transpose(0, 1, 4, 3, 2))  # [c, t, f, k, b]
    xhi = _round_fp32r(xr)
    xlo = (xr - xhi).astype(ml_dtypes.bfloat16)

    shared = dict(w1r=w1r, cpk=cpk)
    return [dict(xt=xhi[c], xlo=xlo[c], **shared) for c in range(n_cores)]


_NC_CACHE = {}


def _get_program(nt, T):
    key = (nt, T)
    if key not in _NC_CACHE:
        _NC_CACHE[key] = build_program(nt, T)
    return _NC_CACHE[key]


def kernel(x, W1, b1, W2, b2, _T=32):
    x = np.asarray(x)
    W1 = np.asarray(W1)
    b1 = np.asarray(b1)
    W2 = np.asarray(W2)
    b2 = np.asarray(b2)
    B = x.shape[0]
    nt = B // (N_CORES * P)
    nc = _get_program(nt, _T)
    in_maps = _prep_host(x, W1, b1, W2, b2, N_CORES, nt)
    res = run_bass_kernel_spmd(nc, in_maps, core_ids=list(range(N_CORES)))
    kernel.last_results = res
    return np.concatenate([res.results[c]["out"] for c in range(N_CORES)], axis=0)


# revision 12
# speedup vs baseline: 86.4293x; 86.4293x over previous
"""Trainium2 Bass kernel for nn_DeeperHyperbolicEncoder.

Math (per batch row r; D_in=512, D_h=256, D_out=128):
  v   = x @ W1^T                 layer-1 matmul (+ fused v.b1 column)
  g   = beta*v + gamma*b1        mobius_add(expmap0(v), b1) collapsed to
                                 per-row scalars from s1=|v|^2, dot=v.b1
  u   = tanh(sb*v + sg*b1)       project+logmap0+tanh folded into row scalars
  q   = u @ W2^T                 (+ fused q.b2 column; mobius_matvec(W2, expmap0(u))
                                 == expmap0(u @ W2^T))
  out = pb*q + pg*b2             mobius_add + double-project via analytic norms

Precision: layer-1 matmul runs as a 3-term fp32r split (x_hi@W_hi + x_hi@W_lo
+ x_lo_bf16@W_bf16) which is exact to ~2^-21; fp32r (11-bit mantissa) streams
at 1 cyc/row vs 4 for fp32. Layer-2 matmul is plain fp32 (its operand u is
produced on device and cannot be cheaply hi/lo split).

Per-row scalar chains are batched across T row-tiles as [128, T] wides.
Data-parallel across 8 NeuronCores (batch split), weights replicated.
"""

import numpy as np
import ml_dtypes

import concourse.bass as bass
import concourse.tile as tile
from concourse import bacc, mybir
from concourse.bass_utils import run_bass_kernel_spmd

F32 = mybir.dt.float32
F32R = mybir.dt.float32r
BF16 = mybir.dt.bfloat16
AF = mybir.ActivationFunctionType
OP = mybir.AluOpType

EPS = 1e-15
MAXN = 1.0 - 4e-3

P = 128
D_IN = 512
D_H = 256
D_OUT = 128
N_CORES = 8


def build_program(nt: int, T: int, reps: int = 1) -> bass.Bass:
    assert nt % T == 0
    n_sb = nt // T

    nc = bacc.Bacc("TRN2", target_bir_lowering=False, debug=False)

    NW = 260   # layer-1 moving width: 256 outputs + dot col + 3 pad (fp32r needs N%4==0)
    NB = 5168  # packed byte-constants per partition

    xt = nc.dram_tensor("xt", [nt, P, 4, P], F32R, kind="ExternalInput").ap()
    xlo = nc.dram_tensor("xlo", [nt, P, 4, P], BF16, kind="ExternalInput").ap()
    w1r = nc.dram_tensor("w1r", [2, 4, P, NW], F32R, kind="ExternalInput").ap()
    cpk = nc.dram_tensor("cpk", [P, NB], mybir.dt.uint8, kind="ExternalInput").ap()
    out = nc.dram_tensor("out", [nt * P, D_OUT], F32, kind="ExternalOutput").ap()

    with tile.TileContext(nc) as tc:
        from contextlib import ExitStack

        with ExitStack() as ctx:
            if reps == 1:
                _body(ctx, tc, nt, T, n_sb, xt, xlo, w1r, cpk, NW, NB, out)
            else:
                with tc.For_i(0, reps, 1):
                    _body(ctx, tc, nt, T, n_sb, xt, xlo, w1r, cpk, NW, NB, out)
    nc.compile()
    return nc


def _body(ctx, tc, nt, T, n_sb, xt, xlo, w1r, cpk, NW, NB, out):
    nc = tc.nc

    cpool = ctx.enter_context(tc.tile_pool(name="cpool", bufs=1))
    w1r_sb = cpool.tile([P, 2, 4, NW], F32R, name="w1r_sb")
    nc.sync.dma_start(w1r_sb[:], w1r.rearrange("h k p n -> p h k n"))
    w1hi_sb = w1r_sb[:, 0]
    w1lo_sb = w1r_sb[:, 1]
    cpk_sb = cpool.tile([P, NB], mybir.dt.uint8, name="cpk_sb")
    nc.sync.dma_start(cpk_sb[:], cpk[:])
    w1b_sb = cpk_sb[:, 0:2080].bitcast(BF16).rearrange("p (k n) -> p k n", k=4)
    w2_sb = cpk_sb[:, 2080:3112].bitcast(F32).rearrange("p (k n) -> p k n", k=2)
    b1_sb = cpk_sb[:, 3112:4136].bitcast(F32)
    b2_sb = cpk_sb[:, 4136:4648].bitcast(F32)
    id_sb = cpk_sb[:, 4648:5160].bitcast(F32)
    cst = cpk_sb[:, 5160:5168].bitcast(F32)
    y1 = cst[:, 0:1]
    y2 = cst[:, 1:2]

    xpool = ctx.enter_context(tc.tile_pool(name="xpool", bufs=3))
    vwpool = ctx.enter_context(tc.tile_pool(name="vwpool", bufs=2))
    qwpool = ctx.enter_context(tc.tile_pool(name="qwpool", bufs=2))
    scpool = ctx.enter_context(tc.tile_pool(name="scpool", bufs=2))
    gpool = ctx.enter_context(tc.tile_pool(name="gpool", bufs=3))
    upool = ctx.enter_context(tc.tile_pool(name="upool", bufs=3))
    utpool = ctx.enter_context(tc.tile_pool(name="utpool", bufs=3))
    opool = ctx.enter_context(tc.tile_pool(name="opool", bufs=4))
    pvpool = ctx.enter_context(tc.tile_pool(name="pvpool", bufs=3, space="PSUM"))
    ptpool = ctx.enter_context(tc.tile_pool(name="ptpool", bufs=2, space="PSUM"))
    pqpool = ctx.enter_context(tc.tile_pool(name="pqpool", bufs=2, space="PSUM"))

    for sb in range(n_sb):
        vw = vwpool.tile([P, T, D_H + 1], F32, name="vw")
        qw = qwpool.tile([P, T, D_OUT + 1], F32, name="qw")
        s1w = scpool.tile([P, T], F32, name="s1w")
        sqw = scpool.tile([P, T], F32, name="sqw")

        # ---------------- phase A: load, mm1 (3-term), evacuate, reduce ----
        for t in range(T):
            ti = sb * T + t
            xsb = xpool.tile([P, 4, P], F32R, name="xsb")
            nc.sync.dma_start(xsb[:], xt[ti])
            xlsb = xpool.tile([P, 4, P], BF16, name="xlsb")
            nc.sync.dma_start(xlsb[:], xlo[ti])
            pv = pvpool.tile([P, NW], F32, name="pv")
            nmm = 0
            for wsb, xop in ((w1hi_sb, xsb), (w1lo_sb, xsb), (w1b_sb, xlsb)):
                for k in range(4):
                    nc.tensor.matmul(
                        pv[:],
                        xop[:, k, :],
                        wsb[:, k, :],
                        start=(nmm == 0),
                        stop=(nmm == 11),
                    )
                    nmm += 1
            nc.scalar.activation(vw[:, t, :], pv[:, : D_H + 1], AF.Copy)
            nc.scalar.activation(
                pv[:, :D_H], pv[:, :D_H], AF.Square, accum_out=s1w[:, t : t + 1]
            )

        # ---------------- chain A: layer-1 per-row scalars -----------------
        dotw = vw[:, :, D_H]

        def st(name):
            return scpool.tile([P, T], F32, name=name)

        n1 = st("n1")
        nc.scalar.activation(n1[:], s1w[:], AF.Sqrt)
        n1c = st("n1c")
        nc.vector.tensor_scalar(n1c[:], n1[:], EPS, None, op0=OP.max)
        rn1 = st("rn1")
        nc.vector.reciprocal(rn1[:], n1c[:])
        th = st("th")
        nc.scalar.activation(th[:], n1c[:], AF.Tanh)
        a1 = st("a1")
        nc.vector.tensor_tensor(a1[:], th[:], rn1[:], op=OP.mult)
        xy = st("xy")
        nc.vector.tensor_tensor(xy[:], a1[:], dotw, op=OP.mult)
        z = st("z")
        nc.vector.tensor_scalar(z[:], xy[:], 2.0, 1.0, op0=OP.mult, op1=OP.add)
        unum = st("unum")
        nc.vector.tensor_scalar(unum[:], z[:], y1, None, op0=OP.add)
        x2 = st("x2")
        nc.vector.tensor_tensor(x2[:], th[:], th[:], op=OP.mult)
        den = st("den")
        nc.vector.scalar_tensor_tensor(den[:], x2[:], y1, z[:], op0=OP.mult, op1=OP.add)
        rden = st("rden")
        nc.vector.reciprocal(rden[:], den[:])
        bta = st("bta")
        nc.vector.tensor_tensor(bta[:], unum[:], rden[:], op=OP.mult)
        beta = st("beta")
        nc.vector.tensor_tensor(beta[:], bta[:], a1[:], op=OP.mult)
        omx2 = st("omx2")
        nc.vector.tensor_scalar(omx2[:], x2[:], -1.0, 1.0, op0=OP.mult, op1=OP.add)
        gam = st("gam")
        nc.vector.tensor_tensor(gam[:], omx2[:], rden[:], op=OP.mult)
        sa = st("sa")
        nc.vector.tensor_tensor(sa[:], beta[:], s1w[:], op=OP.mult)
        sb2 = st("sb2")
        nc.vector.tensor_tensor(sb2[:], gam[:], dotw, op=OP.mult)
        sc_ = st("sc_")
        nc.vector.scalar_tensor_tensor(
            sc_[:], sb2[:], 2.0, sa[:], op0=OP.mult, op1=OP.add
        )
        sd = st("sd")
        nc.vector.tensor_tensor(sd[:], sc_[:], beta[:], op=OP.mult)
        ge = st("ge")
        nc.vector.tensor_tensor(ge[:], gam[:], gam[:], op=OP.mult)
        s2 = st("s2")
        nc.vector.scalar_tensor_tensor(s2[:], ge[:], y1, sd[:], op0=OP.mult, op1=OP.add)
        n2 = st("n2")
        nc.scalar.activation(n2[:], s2[:], AF.Sqrt)
        m_ = st("m_")
        nc.vector.tensor_scalar(m_[:], n2[:], MAXN, None, op0=OP.min)
        rn2 = st("rn2")
        nc.vector.reciprocal(rn2[:], n2[:])
        onep = st("onep")
        nc.vector.tensor_scalar(onep[:], m_[:], 1.0, None, op0=OP.add)
        onem = st("onem")
        nc.vector.tensor_scalar(onem[:], m_[:], -1.0, 1.0, op0=OP.mult, op1=OP.add)
        rom = st("rom")
        nc.vector.reciprocal(rom[:], onem[:])
        rat = st("rat")
        nc.vector.tensor_tensor(rat[:], onep[:], rom[:], op=OP.mult)
        lg = st("lg")
        nc.scalar.activation(lg[:], rat[:], AF.Ln)
        lp = st("lp")
        nc.vector.scalar_tensor_tensor(
            lp[:], lg[:], 0.5, rn2[:], op0=OP.mult, op1=OP.mult
        )
        sbw = st("sbw")
        nc.vector.tensor_tensor(sbw[:], lp[:], beta[:], op=OP.mult)
        sgw = st("sgw")
        nc.vector.tensor_tensor(sgw[:], lp[:], gam[:], op=OP.mult)

        # ---------------- phase B: u = tanh(sb*v + sg*b1); transpose; mm2 --
        for t0 in range(0, T, 2):
            us = []
            for t in (t0, t0 + 1):
                gt = gpool.tile([P, D_H], F32, name="gt")
                nc.vector.tensor_scalar(
                    gt[:], vw[:, t, :D_H], sbw[:, t : t + 1], None, op0=OP.mult
                )
                zt = gpool.tile([P, D_H], F32, name="zt")
                nc.vector.scalar_tensor_tensor(
                    zt[:], b1_sb, sgw[:, t : t + 1], gt[:], op0=OP.mult, op1=OP.add
                )
                ut_ = upool.tile([P, D_H], F32, name="ut_")
                nc.scalar.activation(ut_[:], zt[:], AF.Tanh)
                us.append(ut_)
            ptr = ptpool.tile([P, 4 * P], F32, name="ptr")
            for j, (ui, k) in enumerate([(0, 0), (0, 1), (1, 0), (1, 1)]):
                nc.tensor.transpose(
                    ptr[:, j * P : (j + 1) * P],
                    us[ui][:, k * P : (k + 1) * P],
                    id_sb,
                )
            utt = utpool.tile([P, 4 * P], F32, name="utt")
            nc.vector.tensor_copy(utt[:], ptr[:])
            pq = pqpool.tile([P, 2, D_OUT + 1], F32, name="pq")
            for i in range(2):
                for k in range(2):
                    nc.tensor.matmul(
                        pq[:, i, :],
                        utt[:, (2 * i + k) * P : (2 * i + k + 1) * P],
                        w2_sb[:, k, :],
                        start=(k == 0),
                        stop=(k == 1),
                    )
            nc.vector.tensor_copy(qw[:, t0 : t0 + 2, :], pq[:])
            for i, t in enumerate((t0, t0 + 1)):
                nc.scalar.activation(
                    pq[:, i, :D_OUT],
                    pq[:, i, :D_OUT],
                    AF.Square,
                    accum_out=sqw[:, t : t + 1],
                )

        # ---------------- chain C: layer-2 per-row scalars -----------------
        dot2w = qw[:, :, D_OUT]
        nq = st("nq")
        nc.scalar.activation(nq[:], sqw[:], AF.Sqrt)
        nqc = st("nqc")
        nc.vector.tensor_scalar(nqc[:], nq[:], EPS, None, op0=OP.max)
        rq = st("rq")
        nc.vector.reciprocal(rq[:], nqc[:])
        thq = st("thq")
        nc.scalar.activation(thq[:], nqc[:], AF.Tanh)
        aq = st("aq")
        nc.vector.tensor_tensor(aq[:], thq[:], rq[:], op=OP.mult)
        xy2 = st("xy2")
        nc.vector.tensor_tensor(xy2[:], aq[:], dot2w, op=OP.mult)
        z2 = st("z2")
        nc.vector.tensor_scalar(z2[:], xy2[:], 2.0, 1.0, op0=OP.mult, op1=OP.add)
        unum2 = st("unum2")
        nc.vector.tensor_scalar(unum2[:], z2[:], y2, None, op0=OP.add)
        x22 = st("x22")
        nc.vector.tensor_tensor(x22[:], thq[:], thq[:], op=OP.mult)
        den2 = st("den2")
        nc.vector.scalar_tensor_tensor(
            den2[:], x22[:], y2, z2[:], op0=OP.mult, op1=OP.add
        )
        rden2 = st("rden2")
        nc.vector.reciprocal(rden2[:], den2[:])
        b2a = st("b2a")
        nc.vector.tensor_tensor(b2a[:], unum2[:], rden2[:], op=OP.mult)
        b2c = st("b2c")
        nc.vector.tensor_tensor(b2c[:], b2a[:], aq[:], op=OP.mult)
        omx22 = st("omx22")
        nc.vector.tensor_scalar(omx22[:], x22[:], -1.0, 1.0, op0=OP.mult, op1=OP.add)
        g2c = st("g2c")
        nc.vector.tensor_tensor(g2c[:], omx22[:], rden2[:], op=OP.mult)
        sa2 = st("sa2")
        nc.vector.tensor_tensor(sa2[:], b2c[:], sqw[:], op=OP.mult)
        sb3 = st("sb3")
        nc.vector.tensor_tensor(sb3[:], g2c[:], dot2w, op=OP.mult)
        sc3 = st("sc3")
        nc.vector.scalar_tensor_tensor(
            sc3[:], sb3[:], 2.0, sa2[:], op0=OP.mult, op1=OP.add
        )
        sd2 = st("sd2")
        nc.vector.tensor_tensor(sd2[:], sc3[:], b2c[:], op=OP.mult)
        ge2 = st("ge2")
        nc.vector.tensor_tensor(ge2[:], g2c[:], g2c[:], op=OP.mult)
        np2 = st("np2")
        nc.vector.scalar_tensor_tensor(
            np2[:], ge2[:], y2, sd2[:], op0=OP.mult, op1=OP.add
        )
        npre = st("npre")
        nc.scalar.activation(npre[:], np2[:], AF.Sqrt)
        rnp = st("rnp")
        nc.vector.reciprocal(rnp[:], npre[:])
        pi_ = st("pi_")
        nc.vector.tensor_scalar(pi_[:], rnp[:], MAXN, 1.0, op0=OP.mult, op1=OP.min)
        pb2 = st("pb2")
        nc.vector.tensor_tensor(pb2[:], pi_[:], b2c[:], op=OP.mult)
        pg2 = st("pg2")
        nc.vector.tensor_tensor(pg2[:], pi_[:], g2c[:], op=OP.mult)

        # ---------------- phase D: final combine + store -------------------
        for t in range(T):
            ti = sb * T + t
            o1 = opool.tile([P, D_OUT], F32, name="o1")
            nc.vector.tensor_scalar(
                o1[:], qw[:, t, :D_OUT], pb2[:, t : t + 1], None, op0=OP.mult
            )
            o2 = opool.tile([P, D_OUT], F32, name="o2")
            nc.vector.scalar_tensor_tensor(
                o2[:], b2_sb, pg2[:, t : t + 1], o1[:], op0=OP.mult, op1=OP.add
            )
            nc.sync.dma_start(out[ti * P : (ti + 1) * P, :], o2[:])


def _round_fp32r(a):
    u = np.ascontiguousarray(a, dtype=np.float32).view(np.uint32)
    lsb = (u >> 12) & 1
    rounded = u + 0x7FF + lsb
    return (rounded & 0xFFFFF000).view(np.float32)


def _prep_host(x, W1, b1, W2, b2, n_cores, nt):
    B = x.shape[0]
    assert B == n_cores * nt * P

    W1d = W1.T.astype(np.float64)
    b1d = b1.astype(np.float64)
    W2d = W2.T.astype(np.float64)
    b2d = b2.astype(np.float64)

    NW = 260
    w1ta = np.zeros((D_IN, NW), dtype=np.float32)
    w1ta[:, :D_H] = W1.T.astype(np.float32)
    w1ta[:, D_H] = (W1d @ b1d).astype(np.float32)
    w1hi = _round_fp32r(w1ta)
    w1lo = _round_fp32r(w1ta - w1hi)
    # w1r: [2(hi/lo), 4, P, NW] fp32r
    w1r = np.stack([w1hi.reshape(4, P, NW), w1lo.reshape(4, P, NW)], axis=0)
    w1r = np.ascontiguousarray(w1r)

    # byte-packed constants, laid out per partition: w1b(bf16) | w2tp(f32) |
    # b1f | b2f | ident | [y1, y2]
    w1bf = w1ta.astype(ml_dtypes.bfloat16).reshape(4, P, NW)
    w1bf_p = np.ascontiguousarray(w1bf.transpose(1, 0, 2)).view(np.uint8)
    w1bf_p = w1bf_p.reshape(P, -1)
    w2tp = np.concatenate(
        [W2.T.astype(np.float32), (W2d @ b2d).astype(np.float32)[:, None]], axis=1
    ).reshape(2, P, D_OUT + 1)
    w2tp_p = np.ascontiguousarray(w2tp.transpose(1, 0, 2)).view(np.uint8)
    w2tp_p = w2tp_p.reshape(P, -1)
    b1f = np.ascontiguousarray(np.broadcast_to(b1, (P, D_H)), dtype=np.float32)
    b2f = np.ascontiguousarray(np.broadcast_to(b2, (P, D_OUT)), dtype=np.float32)
    identf = np.eye(P, dtype=np.float32)
    consts = np.zeros((P, 2), dtype=np.float32)
    consts[:, 0] = np.float32(b1d @ b1d)
    consts[:, 1] = np.float32(b2d @ b2d)
    cpk = np.concatenate(
        [
            w1bf_p,
            w2tp_p,
            b1f.view(np.uint8).reshape(P, -1),
            b2f.view(np.uint8).reshape(P, -1),
            identf.view(np.uint8).reshape(P, -1),
            consts.view(np.uint8).reshape(P, -1),
        ],
        axis=1,
    )
    assert cpk.shape == (P, 5168), cpk.shape

    # x -> [core, tile, f(128), k(4), b(128)] transposed blocks; hi in fp32r,
    # residual in bf16
    xr = x.reshape(n_cores, nt, P, 4, P)                   # [c, t, b, k, f]
    xr = np.ascontiguousarray(xr.transpose(0, 1, 4, 3, 2))  # [c, t, f, k, b]
    xhi = _round_fp32r(xr)
    xlo = (xr - xhi).astype(ml_dtypes.bfloat16)

    shared = dict(w1r=w1r, cpk=cpk)
    return [dict(xt=xhi[c], xlo=xlo[c], **shared) for c in range(n_cores)]


_NC_CACHE = {}


def _get_program(nt, T):
    key = (nt, T)
    if key not in _NC_CACHE:
        _NC_CACHE[key] = build_program(nt, T)
    return _NC_CACHE[key]


def kernel(x, W1, b1, W2, b2, _T=32):
    x = np.asarray(x)
    W1 = np.asarray(W1)
    b1 = np.asarray(b1)
    W2 = np.asarray(W2)
    b2 = np.asarray(b2)
    B = x.shape[0]
    nt = B // (N_CORES * P)
    nc = _get_program(nt, _T)
    in_maps = _prep_host(x, W1, b1, W2, b2, N_CORES, nt)
    res = run_bass_kernel_spmd(nc, in_maps, core_ids=list(range(N_CORES)))
    kernel.last_results = res
    return np.concatenate([res.results[c]["out"] for c in range(N_CORES)], axis=0)
